# revision 1
# baseline (speedup 1.0000x reference)
"""MAB (pre-norm multihead attention block) Trainium2 kernel, v2.

Data-parallel over batch: B=8 -> 8 NeuronCores, no collectives.

Key optimizations over v1:
- Host-side compaction of masked K rows (pad_mask zeroes ~half the keys;
  dropping them is exact: reference gives them exp(-1e12)=0 weight and
  multiplies by 0). K-side work (LN, K/V proj, scores, exp, AV) scales
  by n_kt/8 (typically 5/8).
- Host-side transpose of Q and K (bf16) removes all stage-A PE
  transposes + PSUM->SBUF copies. LayerNorm stats are computed on-chip
  from the natural-layout copy, broadcast along the free dim via K=1
  ones-outer-product matmuls, and applied to the transposed tiles with
  2-byte 4x-mode DVE ops. K path is independent of the Q path and runs
  first, so K/V projections (and then attention) start early.
- All matmul operands bf16 (fp32 PSUM accumulate). Halves weight DMA,
  enables 2x/4x DVE modes.
- Fat-tile DMAs (one dma_start per tensor) cut DMA issue overhead from
  ~77 to ~28 instructions.
- Projection biases folded into the PSUM->SBUF evacuation op.
- Attention emission interleaves with Q-projection (pair hp right after
  qpT[hp]) so ACT exp work overlaps the remaining projection matmuls;
  within a pair, AV matmuls interleave with the scores/exp kt loop.
- Explicit engine balance: exp/sqrt on ACT, stats/normalize on DVE,
  residual adds + final norm on GpSimd (Pool), PSUM copies on `any`.

Host-side prep folds LN gains/biases into the weights:
    W*_eff = W* x g_ln[None,:]  (shipped pre-transposed [D_in, D_out], bf16)
    b*_eff = b* + W* @ be_ln
"""

import math
from contextlib import ExitStack

import numpy as np

import concourse.bass as bass
import concourse.tile as tile
from concourse import bacc, mybir
from concourse.masks import make_identity

F32 = mybir.dt.float32
BF16 = mybir.dt.bfloat16
AF = mybir.ActivationFunctionType
ALU = mybir.AluOpType

P = 128
S = 1024          # query sequence length
D = 1024          # model dim
H = 16            # heads
DH = 64           # head dim
NT = S // P       # 8 row tiles
QB = 512          # PSUM bank = 512 fp32
EPS = 1e-5
SCALE = 1.0 / 32.0  # 1/sqrt(D)
NCORES = 8

E_BUFS = 14       # bf16 [128,1024] attention-prob tiles in flight


def _build_nc(nkt, affine=True, zbias=False):
    """Build the kernel for nkt compacted key tiles (SK = nkt*128)."""
    # affine=False omits the final gf/bf application (host detected identity)
    # zbias=True omits the K=1 bias matmuls for V/Wo (host detected zeros)
    SK = nkt * P
    nc = bacc.Bacc("TRN2", target_bir_lowering=False, debug=False)

    qT_h = nc.declare_dram_parameter("qT", [D, S], BF16, isOutput=False)
    q_h = nc.declare_dram_parameter("q", [S, D], BF16, isOutput=False)
    kT_h = nc.declare_dram_parameter("kT", [D, SK], BF16, isOutput=False)
    k_h = nc.declare_dram_parameter("k", [SK, D], BF16, isOutput=False)
    mask_h = nc.declare_dram_parameter("mask", [SK], F32, isOutput=False)
    wq_h = nc.declare_dram_parameter("wq", [D, D], BF16, isOutput=False)
    wk_h = nc.declare_dram_parameter("wk", [D, D], BF16, isOutput=False)
    wv_h = nc.declare_dram_parameter("wv", [D, D], BF16, isOutput=False)
    wo_h = nc.declare_dram_parameter("wo", [D, D], BF16, isOutput=False)
    bcols_h = nc.declare_dram_parameter("bcols", [P, 2 * NT], F32, isOutput=False)
    brows_h = nc.declare_dram_parameter("brows", [2, D], BF16, isOutput=False)
    gf_h = nc.declare_dram_parameter("gf", [D], F32, isOutput=False)
    bf_h = nc.declare_dram_parameter("bf", [D], F32, isOutput=False)
    out_h = nc.declare_dram_parameter("out", [S, D], F32, isOutput=True)

    def bcast_ap(vec_ap, parts=P):
        return bass.AP(tensor=vec_ap.tensor, offset=vec_ap.offset,
                       ap=[[0, parts]] + vec_ap.ap)

    def fat(src_h):
        """[T*P, c] dram tensor -> [P, T*c] fat-tile traversal."""
        return src_h[:, :].rearrange("(t p) s -> p t s", p=P)

    with tile.TileContext(nc) as tc, ExitStack() as ctx:
        persist = ctx.enter_context(tc.tile_pool(name="persist", bufs=1))
        small = ctx.enter_context(tc.tile_pool(name="small", bufs=10))
        psT = ctx.enter_context(tc.tile_pool(name="psT", bufs=2, space="PSUM"))
        psMM = ctx.enter_context(tc.tile_pool(name="psMM", bufs=2, space="PSUM"))
        psAV = ctx.enter_context(tc.tile_pool(name="psAV", bufs=1, space="PSUM"))

        # ---- input DMAs: K path first, weights interleaved by first use ----
        bc_stack = ctx.enter_context(ExitStack())
        knat_pool = bc_stack.enter_context(
            tc.tile_pool(name="knat", side="right", bufs=1))
        kt_pool = bc_stack.enter_context(
            tc.tile_pool(name="ktp", side="right", bufs=1))
        qt_pool = bc_stack.enter_context(
            tc.tile_pool(name="qtp", side="right", bufs=1))
        knat_f = knat_pool.tile([P, nkt * D], BF16, tag="kn", name="knat")
        kT_f = kt_pool.tile([P, NT * SK], BF16, tag="kT", name="kT")
        qT_f = qt_pool.tile([P, NT * S], BF16, tag="qT", name="qT")

        qnat_pool = ctx.enter_context(tc.tile_pool(name="qnat", bufs=1))
        qnat_f = qnat_pool.tile([P, NT * D], BF16, tag="qn", name="qnat")

        ksplit = max(1, nkt // 2)
        nc.sync.dma_start(out=knat_f[:, 0:ksplit * D],
                          in_=k_h[0:ksplit * P, :].rearrange("(t p) s -> p t s", p=P))
        nc.sync.dma_start(out=knat_f[:, ksplit * D:],
                          in_=k_h[ksplit * P:, :].rearrange("(t p) s -> p t s", p=P))
        nc.sync.dma_start(out=kT_f, in_=fat(kT_h))

        k_nat = [knat_f[:, i * D:(i + 1) * D] for i in range(nkt)]
        q_nat = [qnat_f[:, i * D:(i + 1) * D] for i in range(NT)]
        kT = [kT_f[:, i * SK:(i + 1) * SK] for i in range(NT)]
        qT = [qT_f[:, i * S:(i + 1) * S] for i in range(NT)]

        # ---- constants (small DMAs on the ACT HWDGE queue) ----
        ident_f = persist.tile([P, P], F32)
        make_identity(nc, ident_f)
        ident_b = persist.tile([P, P], BF16)
        make_identity(nc, ident_b)
        eps_col = persist.tile([P, 1], F32)
        nc.gpsimd.memset(eps_col, EPS)
        ones_b = persist.tile([1, P], BF16)
        nc.gpsimd.memset(ones_b, 1.0)
        # sel[k, r*P + p] = (k == r): one-hot lhsT blocks; sel[0:n, r*P:(r+1)*P]
        # as matmul lhsT broadcasts statsT row r across all 128 partitions.
        NSEL = max(16, 2 * nkt)
        sel = persist.tile([NSEL, NSEL * P], BF16)
        nc.gpsimd.memset(sel, 0.0)
        # sel[k, (r, p)] = (k - r) != 0 ? 0 : 1  -> one-hot block rows
        nc.gpsimd.affine_select(out=sel, in_=sel,
                                pattern=[[-1, NSEL], [0, P]],
                                compare_op=ALU.not_equal, fill=1.0, base=0,
                                channel_multiplier=1)
        # PE warm-up: the cost model (and HW HAM) run matmuls at half rate
        # for ~3us after the PE goes busy; the PE is otherwise idle until
        # ~10us (K stats chain), so burn that window keeping the ramp warm.
        warm_ps = psT.tile([P, P], F32, tag="ptb", name="warm")
        for _ in range(75):
            nc.tensor.matmul(warm_ps, lhsT=ident_b, rhs=ident_b,
                             start=True, stop=True)
        bv_sb = persist.tile([1, D], BF16)
        nc.scalar.dma_start(out=bv_sb, in_=brows_h[0:1, :])
        bcols = persist.tile([P, 2 * NT], F32)
        nc.scalar.dma_start(out=bcols, in_=bcols_h[:, :])
        # mask -> additive exp bias per k-tile column: (m-1)*1e4
        m_raw = small.tile([P, nkt], F32, tag="mraw", name="m_raw")
        nc.scalar.dma_start(out=m_raw, in_=mask_h[:].rearrange("(t p) -> p t", t=nkt))
        mb = persist.tile([P, nkt], F32)
        nc.vector.tensor_scalar(out=mb, in0=m_raw, scalar1=1.0, scalar2=10000.0,
                                op0=ALU.subtract, op1=ALU.mult)

        opool = ctx.enter_context(tc.tile_pool(name="opool", bufs=NT))
        O_sb = [opool.tile([P, D], BF16, tag="o", name=f"O{i}") for i in range(NT)]

        qpt_pool = bc_stack.enter_context(tc.tile_pool(name="qpt", side="right", bufs=NT))
        kpt_pool = bc_stack.enter_context(tc.tile_pool(name="kpt", side="right", bufs=NT))
        vpa_pool = bc_stack.enter_context(tc.tile_pool(name="vpa", side="right", bufs=nkt))
        e_bufs = E_BUFS if nkt <= 5 else max(2 * nkt, 24 - 2 * nkt)  # SBUF guard
        epool = bc_stack.enter_context(tc.tile_pool(name="epool", side="right",
                                                    bufs=e_bufs))
        otpool = bc_stack.enter_context(tc.tile_pool(name="otpool", side="right",
                                                     bufs=3))
        qpT = [qpt_pool.tile([P, S], BF16, tag="qpt", name=f"qpT{i}") for i in range(NT)]
        kpT = [kpt_pool.tile([P, SK], BF16, tag="kpt", name=f"kpT{i}") for i in range(NT)]
        vpa = [vpa_pool.tile([P, H * (DH + 1)], BF16, tag="vpa", name=f"vpa{i}")
               for i in range(nkt)]

        w_stack = ExitStack()
        wpool = w_stack.enter_context(
            tc.tile_pool(name="wpool", side="right", bufs=1))
        wk_f = wpool.tile([P, NT * D], BF16, tag="wk", name="wk")
        wv_f = wpool.tile([P, NT * D], BF16, tag="wv", name="wv")
        wq_f = wpool.tile([P, NT * D], BF16, tag="wq", name="wq")
        nc.sync.dma_start(out=wk_f[:, 0:4 * D],
                          in_=wk_h[0:4 * P, :].rearrange("(t p) s -> p t s", p=P))
        nc.sync.dma_start(out=wk_f[:, 4 * D:],
                          in_=wk_h[4 * P:, :].rearrange("(t p) s -> p t s", p=P))
        nc.sync.dma_start(out=wv_f, in_=fat(wv_h))
        nc.sync.dma_start(out=qnat_f, in_=fat(q_h))
        nc.sync.dma_start(out=qT_f, in_=fat(qT_h))
        nc.sync.dma_start(out=wq_f, in_=fat(wq_h))
        wk = [wk_f[:, i * D:(i + 1) * D] for i in range(NT)]
        wv = [wv_f[:, i * D:(i + 1) * D] for i in range(NT)]
        wq = [wq_f[:, i * D:(i + 1) * D] for i in range(NT)]

        # ---- stage A: LN stats (natural layout) -> normalize transposed ----
        def ln_stats(x_ap, rstd_ap, negm_ap):
            """bn stats of a [P, D] tile -> rstd and -mean*rstd columns."""
            st6 = small.tile([P, 2, 6], F32, tag="st6", name="st6")
            nc.vector.bn_stats(out=st6[:, 0, :], in_=x_ap[:, 0:512])
            nc.vector.bn_stats(out=st6[:, 1, :], in_=x_ap[:, 512:1024])
            mv = small.tile([P, 2], F32, tag="mv", name="mv")
            nc.vector.bn_aggr(out=mv, in_=st6)
            sd = small.tile([P, 1], F32, tag="sd", name="sd")
            nc.scalar.activation(out=sd, in_=mv[:, 1:2], func=AF.Sqrt, bias=eps_col)
            nc.vector.reciprocal(out=rstd_ap, in_=sd)
            nc.vector.tensor_scalar(
                out=negm_ap, in0=mv[:, 0:1],
                scalar1=rstd_ap, scalar2=-1.0,
                op0=ALU.mult, op1=ALU.mult)

        def stat_bcast(xs, n, bdst):
            """LN stats of tiles xs -> (rstd_b, negm_b) broadcast tiles.

            Engines cannot move data across partitions, so: transpose the
            [P, 2n] stats block on the PE, then broadcast row r across all
            partitions with a one-hot selector lhsT (out = onehot_r^T @
            statsT) -- no DMA involved.
            """
            stats_tile = persist.tile([P, 2 * n], F32)
            for j, x in enumerate(xs):
                ln_stats(x, stats_tile[:, j:j + 1], stats_tile[:, n + j:n + j + 1])
            stT_ps = psT.tile([2 * n, P], F32, tag="ptb", name="stT_ps")
            nc.tensor.transpose(stT_ps, stats_tile, ident_f)
            statsT = persist.tile([2 * n, P], BF16)
            nc.any.tensor_copy(out=statsT, in_=stT_ps)
            for half, dst in enumerate(bdst):
                bc_ps = psAV.tile([P, n * P], F32, tag="av", name="bc_ps")
                for j in range(n):
                    r = half * n + j
                    nc.tensor.matmul(bc_ps[:, j * P:(j + 1) * P],
                                     lhsT=sel[0:2 * n, r * P:(r + 1) * P],
                                     rhs=statsT,
                                     start=True, stop=True)
                nc.any.tensor_copy(out=dst, in_=bc_ps)

        # K path first: unblocks K/V projections (and then attention) early
        rstd_kb = persist.tile([P, SK], BF16)
        negm_kb = persist.tile([P, SK], BF16)
        stat_bcast(k_nat, nkt, (rstd_kb, negm_kb))
        for dt in range(NT):
            eng = nc.vector if dt % 2 == 0 else nc.gpsimd
            eng.tensor_tensor(out=kT[dt], in0=kT[dt], in1=rstd_kb, op=ALU.mult)
            eng.tensor_tensor(out=kT[dt], in0=kT[dt], in1=negm_kb, op=ALU.add)

        # ---- stage B projections + stage C attention, interleaved ----
        def k_proj(vt):
            pk = psMM.tile([P, SK], F32, tag="ps", name="pk")
            for dt in range(NT):
                nc.tensor.matmul(pk[:, 0:QB],
                                 lhsT=wk[dt][:, vt * P:(vt + 1) * P],
                                 rhs=kT[dt][:, 0:QB],
                                 start=(dt == 0), stop=(dt == NT - 1))
                if SK > QB:
                    nc.tensor.matmul(pk[:, QB:SK],
                                     lhsT=wk[dt][:, vt * P:(vt + 1) * P],
                                     rhs=kT[dt][:, QB:SK],
                                     start=(dt == 0), stop=(dt == NT - 1))
            nc.any.tensor_scalar(out=kpT[vt], in0=pk,
                                 scalar1=bcols[:, NT + vt:NT + vt + 1],
                                 scalar2=0.0, op0=ALU.add, op1=ALU.add)

        def q_proj(vt):
            # accumulates in two 1-bank psT tiles: keeps the psMM rotation
            # free for attention score tiles (ACT exp never starves)
            pqs = [psT.tile([P, QB], F32, tag="ptb", name=f"pq{qb}")
                   for qb in range(2)]
            for dt in range(NT):
                for qb in range(2):
                    nc.tensor.matmul(pqs[qb],
                                     lhsT=wq[dt][:, vt * P:(vt + 1) * P],
                                     rhs=qT[dt][:, qb * QB:(qb + 1) * QB],
                                     start=(dt == 0), stop=(dt == NT - 1))
            for qb in range(2):
                nc.any.tensor_scalar(out=qpT[vt][:, qb * QB:(qb + 1) * QB],
                                     in0=pqs[qb],
                                     scalar1=bcols[:, vt:vt + 1],
                                     scalar2=0.0, op0=ALU.add, op1=ALU.add)

        def v_proj(kts):
            # out[s_k, v] with interleaved ones column (softmax denom)
            for kt in kts:
                nc.gpsimd.memset(vpa[kt], 1.0)
            for qb in range(2):
                for kt in kts:
                    pv = psMM.tile([P, QB], F32, tag="ps", name="pv")
                    for dt in range(NT):
                        nc.tensor.matmul(pv,
                                         lhsT=kT[dt][:, kt * P:(kt + 1) * P],
                                         rhs=wv[dt][:, qb * QB:(qb + 1) * QB],
                                         start=(dt == 0),
                                         stop=(zbias and dt == NT - 1))
                    if not zbias:
                        nc.tensor.matmul(pv, lhsT=ones_b,
                                         rhs=bv_sb[0:1, qb * QB:(qb + 1) * QB],
                                         start=False, stop=True)
                    nc.scalar.copy(
                        out=vpa[kt].rearrange("p (h x) -> p h x", x=DH + 1)[
                            :, qb * 8:(qb + 1) * 8, 0:DH],
                        in_=pv.rearrange("p (h x) -> p h x", x=DH))

        def scores_exp(hp, par):
            """Scores+exp for one head; returns the e tiles."""
            po = par * DH
            es = []
            for kt in range(nkt):
                sps = psMM.tile([P, S], F32, tag="ps", name="sps")
                for qb in range(2):
                    nc.tensor.matmul(
                        sps[:, qb * QB:(qb + 1) * QB],
                        lhsT=kpT[hp][po:po + DH, kt * P:(kt + 1) * P],
                        rhs=qpT[hp][po:po + DH, qb * QB:(qb + 1) * QB])
                e = epool.tile([P, S], BF16, tag="et", name="e")
                nc.scalar.activation(out=e, in_=sps, func=AF.Exp,
                                     bias=mb[:, kt:kt + 1], scale=SCALE)
                es.append(e)
            return es

        def av_head(h, es):
            avp = psAV.tile([DH + 1, S], F32, tag="av", name="avp")
            for kt in range(nkt):
                for qb in range(2):
                    nc.tensor.matmul(
                        avp[:, qb * QB:(qb + 1) * QB],
                        lhsT=vpa[kt][:, h * (DH + 1):(h + 1) * (DH + 1)],
                        rhs=es[kt][:, qb * QB:(qb + 1) * QB],
                        start=(kt == 0), stop=(kt == nkt - 1))
            ot = otpool.tile([DH + 1, S], BF16, tag="ot", name="ot")
            nc.vector.tensor_copy(out=ot, in_=avp)
            for qt in range(NT):
                pt = psT.tile([P, DH + 1], BF16, tag="ptb", name="ptv")
                nc.tensor.transpose(
                    pt, ot[:, qt * P:(qt + 1) * P], ident_b[0:DH + 1, 0:DH + 1])
                rcp = small.tile([P, 1], F32, tag="rcp", name="rcp")
                nc.vector.reciprocal(rcp, pt[:, DH:DH + 1])
                nc.vector.tensor_scalar(
                    out=O_sb[qt][:, h * DH:(h + 1) * DH],
                    in0=pt[:, 0:DH], scalar1=rcp, scalar2=0.0,
                    op0=ALU.mult, op1=ALU.add)

        from collections import deque
        pending = deque()

        def avout_head(ot, h):
            SL = DH + 2  # 66 bf16 = 132 B per slot keeps PSUM offsets 4B-aligned
            pt = psT.tile([P, NT * SL], BF16, tag="ptb", name="ptv")
            for qt in range(NT):
                nc.tensor.transpose(
                    pt[:, qt * SL:qt * SL + DH + 1],
                    ot[:, qt * P:(qt + 1) * P], ident_b[0:DH + 1, 0:DH + 1])
            for qt in range(NT):
                o = qt * SL
                rcp = small.tile([P, 1], F32, tag="rcp", name="rcp")
                nc.vector.reciprocal(rcp, pt[:, o + DH:o + DH + 1])
                nc.vector.tensor_scalar(
                    out=O_sb[qt][:, h * DH:(h + 1) * DH],
                    in0=pt[:, o:o + DH], scalar1=rcp, scalar2=0.0,
                    op0=ALU.mult, op1=ALU.add)

        def scores_av(hp, par):
            """Scores+exp+AV for one head, AV interleaved per kt.
            The previous head's transpose/scale items drip in two per kt so
            the in-order PE queue never clogs on the DVE epilogue rotation."""
            h = 2 * hp + par
            po = par * DH
            avp = psAV.tile([DH + 1, S], F32, tag="av", name="avp")
            for kt in range(nkt):
                sps = psMM.tile([P, S], F32, tag="ps", name="sps")
                for qb in range(2):
                    nc.tensor.matmul(
                        sps[:, qb * QB:(qb + 1) * QB],
                        lhsT=kpT[hp][po:po + DH, kt * P:(kt + 1) * P],
                        rhs=qpT[hp][po:po + DH, qb * QB:(qb + 1) * QB])
                e = epool.tile([P, S], BF16, tag="et", name="e")
                nc.scalar.activation(out=e, in_=sps, func=AF.Exp,
                                     bias=mb[:, kt:kt + 1], scale=SCALE)
                # AV for this kt: PE work while ACT exps the next tile
                for qb in range(2):
                    nc.tensor.matmul(
                        avp[:, qb * QB:(qb + 1) * QB],
                        lhsT=vpa[kt][:, h * (DH + 1):(h + 1) * (DH + 1)],
                        rhs=e[:, qb * QB:(qb + 1) * QB],
                        start=(kt == 0), stop=(kt == nkt - 1))
            ot = otpool.tile([DH + 1, S], BF16, tag="ot", name="ot")
            nc.vector.tensor_copy(out=ot, in_=avp)
            return ot, h

        for vt in range(NT):
            k_proj(vt)
        v_proj(range(0, 1))
        # Q-path stats/normalize emitted mid-V-proj: late enough that the
        # Q stats are done (no PE wait-queue clog), early enough that the
        # broadcast+normalize+q_proj chain overlaps V-proj's second half.
        rstd_qb = persist.tile([P, S], BF16)
        negm_qb = persist.tile([P, S], BF16)
        stat_bcast(q_nat, NT, (rstd_qb, negm_qb))
        for dt in range(NT):
            eng = nc.vector if dt % 2 == 0 else nc.gpsimd
            eng.tensor_tensor(out=qT[dt], in0=qT[dt], in1=rstd_qb, op=ALU.mult)
            eng.tensor_tensor(out=qT[dt], in0=qT[dt], in1=negm_qb, op=ALU.add)
        v_proj(range(1, nkt))
        q_proj(0)
        q_proj(1)
        for hp in range(NT):
            ot, h = scores_av(hp, 0)
            if pending:
                avout_head(*pending.popleft())
            pending.append((ot, h))
            if hp + 2 < NT:
                q_proj(hp + 2)
            ot, h = scores_av(hp, 1)
            if pending:
                avout_head(*pending.popleft())
            pending.append((ot, h))
        while pending:
            avout_head(*pending.popleft())

        for st in range(NT):
            nc.gpsimd.tensor_tensor(out=O_sb[st], in0=O_sb[st],
                                    in1=q_nat[st], op=ALU.add)
        w_stack.close()
        bc_stack.close()  # free qT/kT/qpT/kpT/vpa/e buffers

        # ---- stage D: residual + LN + FC(relu) + residual + final LN ----
        with tc.tile_pool(name="onp", bufs=2) as onp, \
             tc.tile_pool(name="ontp", bufs=NT) as ontp, \
             tc.tile_pool(name="wop", bufs=1) as wop, \
             tc.tile_pool(name="fin", bufs=1) as fin, \
             tc.tile_pool(name="zp", bufs=3) as zp:
            wo_f = wop.tile([P, NT * D], BF16, tag="wo", name="wo")
            nc.sync.dma_start(out=wo_f, in_=fat(wo_h))
            wo = [wo_f[:, i * D:(i + 1) * D] for i in range(NT)]
            bo_sb = fin.tile([1, D], BF16)
            nc.scalar.dma_start(out=bo_sb, in_=brows_h[1:2, :])
            if affine:
                gf_bc = fin.tile([P, D], F32)
                nc.scalar.dma_start(out=gf_bc, in_=bcast_ap(gf_h[:]))
                bf_bc = fin.tile([P, D], F32)
                nc.scalar.dma_start(out=bf_bc, in_=bcast_ap(bf_h[:]))

            onT = [ontp.tile([P, S], BF16, tag="ont", name=f"onT{i}")
                   for i in range(NT)]
            for st in range(NT):
                s2 = small.tile([P, 2], F32, tag="s2d", name="s2d")
                ln_stats(O_sb[st], s2[:, 0:1], s2[:, 1:2])
                on = onp.tile([P, D], BF16, tag="on", name="on")
                nc.vector.tensor_scalar(
                    out=on, in0=O_sb[st],
                    scalar1=s2[:, 0:1], scalar2=s2[:, 1:2],
                    op0=ALU.mult, op1=ALU.add)
                for dt in range(NT):
                    ptd = psT.tile([P, P], BF16, tag="ptb", name="ptd")
                    nc.tensor.transpose(ptd, on[:, dt * P:(dt + 1) * P], ident_b)
                    nc.any.tensor_copy(out=onT[dt][:, st * P:(st + 1) * P], in_=ptd)

            for st in range(NT):
                pz = psMM.tile([P, S], F32, tag="ps", name="pz")
                for dt in range(NT):
                    for qb in range(2):
                        nc.tensor.matmul(pz[:, qb * QB:(qb + 1) * QB],
                                         lhsT=onT[dt][:, st * P:(st + 1) * P],
                                         rhs=wo[dt][:, qb * QB:(qb + 1) * QB],
                                         start=(dt == 0),
                                         stop=(zbias and dt == NT - 1))
                if not zbias:
                    for qb in range(2):  # K=1 bias row: += ones^T x bo
                        nc.tensor.matmul(pz[:, qb * QB:(qb + 1) * QB],
                                         lhsT=ones_b,
                                         rhs=bo_sb[0:1, qb * QB:(qb + 1) * QB],
                                         start=False, stop=True)
                z = zp.tile([P, D], BF16, tag="z", name="z")
                nc.scalar.activation(out=z, in_=pz, func=AF.Relu)
                nc.gpsimd.tensor_tensor(out=z, in0=z, in1=O_sb[st],
                                        op=ALU.add)
                s2z = small.tile([P, 2], F32, tag="s2z", name="s2z")
                ln_stats(z, s2z[:, 0:1], s2z[:, 1:2])
                zn = zp.tile([P, D], F32, tag="zn", name="zn")
                nc.vector.tensor_scalar(
                    out=zn, in0=z,
                    scalar1=s2z[:, 0:1], scalar2=s2z[:, 1:2],
                    op0=ALU.mult, op1=ALU.add)
                if affine:
                    nc.gpsimd.tensor_tensor(out=zn, in0=zn, in1=gf_bc,
                                            op=ALU.mult)
                    nc.gpsimd.tensor_tensor(out=zn, in0=zn, in1=bf_bc,
                                            op=ALU.add)
                nc.sync.dma_start(out=out_h[st * P:(st + 1) * P, :], in_=zn)

    nc.compile()
    return nc


_NC_CACHE = {}


def _get_nc(nkt, affine=True, zbias=False):
    key = (nkt, affine, zbias)
    if key not in _NC_CACHE:
        _NC_CACHE[key] = _build_nc(nkt, affine, zbias)
    return _NC_CACHE[key]


def _host_prep(inputs):
    import ml_dtypes
    bf16 = ml_dtypes.bfloat16

    f = lambda k: np.asarray(inputs[k], np.float32)
    Q, K, pm = f("Q"), f("K"), f("pad_mask")
    Wq, Wk, Wv, Wo = f("Wq"), f("Wk"), f("Wv"), f("Wo")
    bq, bk, bv, bo = f("bq"), f("bk"), f("bv"), f("bo")
    g_q, be_q = f("g_q"), f("be_q")
    g_kv, be_kv = f("g_kv"), f("be_kv")
    g_o, be_o = f("g_o"), f("be_o")
    g_f, be_f = f("g_f"), f("be_f")

    wq = np.ascontiguousarray((Wq * g_q[None, :]).T).astype(bf16)
    wk = np.ascontiguousarray((Wk * g_kv[None, :]).T).astype(bf16)
    wv = np.ascontiguousarray((Wv * g_kv[None, :]).T).astype(bf16)
    wo = np.ascontiguousarray((Wo * g_o[None, :]).T).astype(bf16)
    bq_eff = (bq + Wq @ be_q).astype(np.float32)
    bk_eff = (bk + Wk @ be_kv).astype(np.float32)
    bv_eff = (bv + Wv @ be_kv).astype(np.float32)
    bo_eff = (bo + Wo @ be_o).astype(np.float32)
    bcols = np.concatenate([bq_eff.reshape(NT, P).T, bk_eff.reshape(NT, P).T],
                           axis=1).astype(np.float32)  # [P, 16]
    brows = np.stack([bv_eff, bo_eff]).astype(bf16)

    n_valid = pm.sum(axis=1).astype(int)
    nkt = max(1, math.ceil(n_valid.max() / P))
    SK = nkt * P
    affine = not (np.all(g_f == 1.0) and np.all(be_f == 0.0))
    zbias = bool(np.all(bv_eff == 0.0) and np.all(bo_eff == 0.0))

    shared = {"wq": wq, "wk": wk, "wv": wv, "wo": wo,
              "bcols": bcols, "brows": brows,
              "gf": g_f, "bf": be_f}
    _host_prep.affine = affine
    _host_prep.zbias = zbias
    in_maps = []
    for i in range(NCORES):
        idx = np.nonzero(pm[i])[0]
        n = len(idx)
        Kc = np.zeros((SK, D), np.float32)
        Kc[:n] = K[i][idx]
        mask_c = np.zeros((SK,), np.float32)
        mask_c[:n] = 1.0
        in_maps.append(dict(
            shared,
            q=np.ascontiguousarray(Q[i]).astype(bf16),
            qT=np.ascontiguousarray(Q[i].T).astype(bf16),
            k=Kc.astype(bf16),
            kT=np.ascontiguousarray(Kc.T).astype(bf16),
            mask=mask_c,
        ))
    return nkt, in_maps


LAST_RESULTS = None


def kernel(**inputs):
    from concourse.bass_utils import run_bass_kernel_spmd

    global LAST_RESULTS
    nkt, in_maps = _host_prep(inputs)
    nc = _get_nc(nkt, _host_prep.affine, _host_prep.zbias)
    res = run_bass_kernel_spmd(nc, in_maps, core_ids=list(range(NCORES)))
    LAST_RESULTS = res
    return np.stack([res.results[i]["out"] for i in range(NCORES)]).astype(np.float32)

